# revision 4
# baseline (speedup 1.0000x reference)
"""Trainium2 Bass kernel for nn_MultiHeadAttnCoupling — final (v10).

HW-measured facts driving this design:
  - Strided MOVING operand: ~4-5 cyc/col (vs 1 contiguous) -> all matmul
    rhs/lhsT operands must be contiguous slices.
  - Strided (sub-16B-run) SBUF WRITES: ~4.3 cyc/elem RMW penalty -> all
    engine-op destinations must be contiguous runs; strided READS are ~1x.
  - Small contiguous MMs (64x64) with per-MM LDW sustain ~37-56 ns on the
    diagonal-quadrant pairing; LDW hides under MMs when operands contiguous.
  - GpSimd strided-read copy: ~5 cyc/elem (slow but a free engine).

Structure:
  ph1 projections: s-major paired slabs, contiguous evictions
      (4x [64,256] per chunk: ACT unshifted pair, DVE shifted pair),
      PSUM from a single 8-bank rotation.
  ph2 attention (groups of G=7 token pairs): per-group repack of K/Q/V
      into contiguous per-pair tiles (ACT=K, DVE=Q, GPS=V; strided read,
      contiguous write), diagonal-quadrant score/AV matmuls, one exp, one
      reciprocal, one fused normalize into the persistent opn_all slab.
  ph3 output projection: chunk-major rearrange opn_all -> otp (4 contiguous
      [64,256] copies per chunk, ACT unshifted / DVE shifted) pipelined
      with the accumulating N=512 output matmuls.
"""

import numpy as np
import ml_dtypes

B, N = 4, 128
INPUT_SIZE, Z_SIZE = 512, 256
DT, H, S = 64, 8, 64
D = DT * H * S            # 32768
DH = S * DT               # 4096 per head
T = B * N                 # 512 tokens
NP = T // 2               # 256 token pairs (j, j+256)
CH = DH // 128            # 32 chunks per projection
KCQ = Z_SIZE // 128       # 2
KCX = INPUT_SIZE // 128   # 4
CT = INPUT_SIZE // 128    # 4 output col tiles
G = 7                     # pairs per attention group (PSUM bank: 7*65<=512)
CB = 4                    # chunks per weight DMA batch

_bf16 = ml_dtypes.bfloat16

_cache = {}


def _build_nc():
    import concourse.mybir as mybir
    import concourse.tile as tile
    from concourse import bacc

    f32, bf16 = mybir.dt.float32, mybir.dt.bfloat16
    AF = mybir.ActivationFunctionType
    MUL = mybir.AluOpType.mult

    nc = bacc.Bacc("TRN2", target_bir_lowering=False, debug=False)

    zt_d = nc.dram_tensor("zt", [128, KCQ, T], bf16, kind="ExternalInput")
    xt_d = nc.dram_tensor("xt", [128, KCX, T], bf16, kind="ExternalInput")
    wq_d = nc.dram_tensor("wq", [128, CH, KCQ * 128], bf16, kind="ExternalInput")
    wk_d = nc.dram_tensor("wk", [128, CH, KCX * 128], bf16, kind="ExternalInput")
    wv_d = nc.dram_tensor("wv", [128, CH, KCX * 128], bf16, kind="ExternalInput")
    wo_d = nc.dram_tensor("wo", [128, CH, CT * 128], bf16, kind="ExternalInput")
    bq_d = nc.dram_tensor("bq", [128, CH], f32, kind="ExternalInput")
    bk_d = nc.dram_tensor("bk", [128, CH], f32, kind="ExternalInput")
    bv_d = nc.dram_tensor("bv", [128, CH], f32, kind="ExternalInput")
    pt_d = nc.dram_tensor("pt", [INPUT_SIZE, T], f32, kind="ExternalOutput")

    HP = T // 2  # 256: token-half size

    with tile.TileContext(nc) as tc:
        with (
            tc.tile_pool(name="acts", bufs=1) as acts_pool,
            tc.tile_pool(name="slabs", bufs=1) as slab_pool,
            tc.tile_pool(name="wts", bufs=3) as wts_pool,
            tc.tile_pool(name="small", bufs=3) as small_pool,
            tc.tile_pool(name="osb", bufs=2) as osb_pool,
            tc.tile_pool(name="ps8", bufs=8, space="PSUM") as ps8,
        ):
            # resident activations and biases
            zt = acts_pool.tile([128, KCQ, T], bf16, tag="zt")
            xt = acts_pool.tile([128, KCX, T], bf16, tag="xt")
            nc.sync.dma_start(zt[:], zt_d[:])
            nc.sync.dma_start(xt[:], xt_d[:])
            bq = acts_pool.tile([128, CH], f32, tag="bq")
            bk = acts_pool.tile([128, CH], f32, tag="bk")
            bv = acts_pool.tile([128, CH], f32, tag="bv")
            nc.sync.dma_start(bq[:], bq_d[:])
            nc.sync.dma_start(bk[:], bk_d[:])
            nc.sync.dma_start(bv[:], bv_d[:])

            # s-major paired slabs: partitions 0:63 <-> token j,
            # 64:127 <-> token j+256
            qts2 = slab_pool.tile([128, S, NP], bf16, tag="qts2")
            kts2 = slab_pool.tile([128, S, NP], bf16, tag="kts2")
            vs2 = slab_pool.tile([128, DT + 1, NP], bf16, tag="vs2")
            NG = (NP + G - 1) // G
            opn_b = slab_pool.tile([128, NG, DT, G], bf16, tag="opn")
            otp = slab_pool.tile([128, CH, T], bf16, tag="otp")
            nc.vector.memset(vs2[:, DT, :], 1.0)

            # ---- ph1: projections ----
            for (w_d, wtag, nkc, act, bias, slab) in (
                (wq_d, "wq", KCQ, zt, bq, qts2),
                (wk_d, "wk", KCX, xt, bk, kts2),
                (wv_d, "wv", KCX, xt, bv, vs2),
            ):
                for c4 in range(CH // CB):
                    wtf = wts_pool.tile([128, CB, KCX, 128], bf16, tag="w",
                                        name=f"wt{wtag}{c4}")
                    wt = wtf[:, :, 0:nkc, :]
                    nc.sync.dma_start(
                        wt[:], w_d[:, CB * c4:CB * (c4 + 1), :].rearrange(
                            "p c (kc m) -> p c kc m", m=128))
                    for ci in range(CB):
                        c = CB * c4 + ci
                        ps = ps8.tile([128, T], f32, tag="b",
                                      name=f"pj{wtag}{c}")
                        for kc in range(nkc):
                            nc.tensor.matmul(
                                ps[:], wt[:, ci, kc, :], act[:, kc, :],
                                start=(kc == 0), stop=(kc == nkc - 1))
                        nc.scalar.activation(
                            slab[0:64, 2 * c, :], ps[0:64, 0:HP],
                            AF.Identity, bias=bias[0:64, c:c + 1])
                        nc.vector.tensor_scalar_add(
                            slab[64:128, 2 * c, :], ps[0:64, HP:T],
                            bias[0:64, c:c + 1])
                        nc.vector.tensor_scalar_add(
                            slab[0:64, 2 * c + 1, :], ps[64:128, 0:HP],
                            bias[64:128, c:c + 1])
                        nc.scalar.activation(
                            slab[64:128, 2 * c + 1, :], ps[64:128, HP:T],
                            AF.Identity, bias=bias[64:128, c:c + 1])

            # ---- ph2: attention, groups of G token pairs ----
            for g in range(NG):
                j0 = g * G
                gp = min(G, NP - j0)
                jsl = slice(j0, j0 + gp)
                kg = small_pool.tile([128, G, S], bf16, tag="kg")
                qg = small_pool.tile([128, G, S], bf16, tag="qg")
                vg = small_pool.tile([128, G, DT + 1], bf16, tag="vg")
                if g % 3 == 2:
                    nc.vector.tensor_copy(vg[:, 0:gp, :],
                                          vs2[:, :, jsl].transpose([0, 2, 1]))
                else:
                    nc.gpsimd.tensor_copy(vg[:, 0:gp, :],
                                          vs2[:, :, jsl].transpose([0, 2, 1]))
                nc.scalar.copy(kg[:, 0:gp, :],
                               kts2[:, :, jsl].transpose([0, 2, 1]))
                nc.vector.tensor_copy(qg[:, 0:gp, :],
                                      qts2[:, :, jsl].transpose([0, 2, 1]))
                sc = ps8.tile([128, G, S], f32, tag="b", name=f"sc{g}")
                for jj in range(gp):
                    nc.tensor.matmul(sc[0:64, jj, :], kg[0:64, jj, :],
                                     qg[0:64, jj, :], start=True, stop=True)
                    nc.tensor.matmul(sc[64:128, jj, :], kg[64:128, jj, :],
                                     qg[64:128, jj, :], start=True, stop=True)
                eT = small_pool.tile([128, G, S], bf16, tag="eT")
                nc.scalar.activation(eT[:, 0:gp, :], sc[:, 0:gp, :], AF.Exp)
                op = ps8.tile([128, G, DT + 1], f32, tag="b", name=f"op{g}")
                for jj in range(gp):
                    nc.tensor.matmul(op[0:64, jj, :], eT[0:64, jj, :],
                                     vg[0:64, jj, :], start=True, stop=True)
                    nc.tensor.matmul(op[64:128, jj, :], eT[64:128, jj, :],
                                     vg[64:128, jj, :], start=True, stop=True)
                rd = small_pool.tile([128, G], f32, tag="rd")
                nc.vector.reciprocal(rd[:, 0:gp], op[:, 0:gp, DT])
                rdb = rd[:, 0:gp].unsqueeze(1).broadcast_to([128, DT, gp])
                nc.vector.tensor_tensor(
                    opn_b[:, g, :, 0:gp],
                    op[:, 0:gp, 0:DT].transpose([0, 2, 1]), rdb, MUL)

            # ---- ph3: chunk-major rearrange + output projection ----
            # one-time tail-group (4 pairs) copies across all chunks
            FB = 36 * G  # 252 full-group tokens
            TL = NP - FB  # 4 tail pairs
            nc.scalar.copy(otp[0:64, :, FB:HP],
                           opn_b[0:64, 36, 0:DT:2, 0:TL])
            nc.vector.tensor_copy(otp[64:128, :, FB:HP],
                                  opn_b[0:64, 36, 1:DT:2, 0:TL])
            nc.vector.tensor_copy(otp[0:64, :, HP + FB:T],
                                  opn_b[64:128, 36, 0:DT:2, 0:TL])
            nc.scalar.copy(otp[64:128, :, HP + FB:T],
                           opn_b[64:128, 36, 1:DT:2, 0:TL])
            fins = [ps8.tile([128, T], f32, tag="b", name=f"fin{i}")
                    for i in range(CT)]
            for cc4 in range(CH // CB):
                wt = wts_pool.tile([128, CB, CT, 128], bf16, tag="w")
                nc.sync.dma_start(
                    wt[:], wo_d[:, CB * cc4:CB * (cc4 + 1), :].rearrange(
                        "p c (ct m) -> p c ct m", m=128))
                for ci in range(CB):
                    cc = CB * cc4 + ci
                    nc.scalar.copy(
                        otp[0:64, cc, 0:FB].rearrange("p (g j) -> p g j", j=G),
                        opn_b[0:64, 0:36, 2 * cc, :])
                    nc.vector.tensor_copy(
                        otp[64:128, cc, 0:FB].rearrange("p (g j) -> p g j", j=G),
                        opn_b[0:64, 0:36, 2 * cc + 1, :])
                    nc.vector.tensor_copy(
                        otp[0:64, cc, HP:HP + FB].rearrange("p (g j) -> p g j", j=G),
                        opn_b[64:128, 0:36, 2 * cc, :])
                    nc.scalar.copy(
                        otp[64:128, cc, HP:HP + FB].rearrange("p (g j) -> p g j", j=G),
                        opn_b[64:128, 0:36, 2 * cc + 1, :])
                    for ct in range(CT):
                        nc.tensor.matmul(
                            fins[ct][:], wt[:, ci, ct, :],
                            otp[:, cc, :],
                            start=(cc == 0), stop=(cc == CH - 1))
            for ct in range(CT):
                ob = osb_pool.tile([128, T], f32, tag="ob")
                nc.vector.tensor_copy(ob[:], fins[ct][:])
                nc.sync.dma_start(pt_d[128 * ct:128 * (ct + 1), :], ob[:])

    nc.compile()
    return nc


# dt-major permutation: new index dt*S+s  <- old index s*DT+dt
_PERM = np.arange(S * DT).reshape(S, DT).T.reshape(-1)


def _prep_core_inputs(h, x, z, Wq, bq, Wk, bk, Wv, bv, Wo):
    dsl = slice(h * DH, (h + 1) * DH)

    def dev_w(w, nkc):
        # [nkc*128, DH] -> [p, c, kc*128+m]
        return np.ascontiguousarray(
            w.reshape(nkc, 128, CH, 128).transpose(1, 2, 0, 3)
            .reshape(128, CH, nkc * 128).astype(_bf16))

    wq_h = Wq[:, dsl] * np.float32(0.125)
    bq_h = bq[dsl] * np.float32(0.125)
    wk_h = Wk[:, dsl]
    bk_h = bk[dsl]
    wv_h = Wv[:, dsl][:, _PERM]
    bv_h = bv[dsl][_PERM]
    wo_h = Wo[dsl, :][_PERM, :]

    zt = z.reshape(T, Z_SIZE).T.reshape(KCQ, 128, T).transpose(1, 0, 2)
    xt = x.reshape(T, INPUT_SIZE).T.reshape(KCX, 128, T).transpose(1, 0, 2)
    return {
        "zt": np.ascontiguousarray(zt.astype(_bf16)),
        "xt": np.ascontiguousarray(xt.astype(_bf16)),
        "wq": dev_w(wq_h, KCQ),
        "wk": dev_w(wk_h, KCX),
        "wv": dev_w(wv_h, KCX),
        "wo": np.ascontiguousarray(
            wo_h.reshape(CH, 128, CT, 128).transpose(1, 0, 2, 3)
            .reshape(128, CH, CT * 128).astype(_bf16)),
        "bq": np.ascontiguousarray(bq_h.reshape(CH, 128).T.astype(np.float32)),
        "bk": np.ascontiguousarray(bk_h.reshape(CH, 128).T.astype(np.float32)),
        "bv": np.ascontiguousarray(bv_h.reshape(CH, 128).T.astype(np.float32)),
    }


def make_in_maps(x, z, Wq, bq, Wk, bk, Wv, bv, Wo):
    x = np.asarray(x, np.float32)
    z = np.asarray(z, np.float32)
    return [
        _prep_core_inputs(h, x, z, np.asarray(Wq, np.float32),
                          np.asarray(bq, np.float32), np.asarray(Wk, np.float32),
                          np.asarray(bk, np.float32), np.asarray(Wv, np.float32),
                          np.asarray(bv, np.float32), np.asarray(Wo, np.float32))
        for h in range(H)
    ]


def get_nc():
    if "nc" not in _cache:
        _cache["nc"] = _build_nc()
    return _cache["nc"]


def run_spmd(in_maps, trace=False):
    from concourse.bass_utils import run_bass_kernel_spmd
    nc = get_nc()
    return run_bass_kernel_spmd(nc, in_maps, list(range(H)), trace=trace)


def assemble_output(results, bo):
    total = np.zeros((INPUT_SIZE, T), np.float64)
    for r in results:
        total += r["pt"].astype(np.float64)
    out = total.T.astype(np.float32) + np.asarray(bo, np.float32)
    return np.ascontiguousarray(out.reshape(B, N, INPUT_SIZE))


def kernel(x, z, Wq, bq, Wk, bk, Wv, bv, Wo, bo):
    in_maps = make_in_maps(x, z, Wq, bq, Wk, bk, Wv, bv, Wo)
    res = run_spmd(in_maps)
    return assemble_output(res.results, bo)
